# revision 31
# baseline (speedup 1.0000x reference)
"""Multi-head attention (B=1, L=2048, D=1024, H=16) on 8 TRN2 NeuronCores.

Sharding: tensor-parallel over heads. Core i computes heads 2i, 2i+1:
projections with column shards of w_q/w_k/w_v, full attention for its 2
heads, partial output projection with the matching 128-row shard of w_o.
Host sums the 8 partial outputs (row-split w_concat => partial-sum).

Redesign vs baseline (236us):
  - Uniform PE config in the hot loop: every matmul is K=128 (or tiny
    K=1/2 outside the loop), M<=128, tile_position (0,0). The baseline's
    per-head K=64/M=64 matmuls with alternating tile_position quadrants
    ran the PE at ~half rate (420-560ns per 512-row matmul vs 216ns).
    Heads are isolated by zero-padding: khT_A has head-B partition rows
    zeroed (so K=128 score matmuls don't mix heads), vh_A has head-B
    columns zeroed (so both heads' AV accumulate into one [128, L/2]
    PSUM tile without cross-talk).
  - Softmax denominators accumulated on the otherwise-idle Vector engine
    (bf16 adds of the exp tiles), column-summed with 4 small matmuls per
    half; normalize is folded into one selector matmul (K=1 pair) plus a
    DVE multiply on the unnormalized AV output.
  - exp (the Scalar-engine wall: 64 x [128,1024] tiles ~ 64us) runs on
    Scalar exclusively; all copies/bias-adds live on Vector/GpSimd.
  - b_o added on device by core 0 only, fused into the GpSimd PSUM->SBUF
    output copies (bo_bc input is zeros on other cores).
  - Input DMA split into 512-column chunks ordered k0,q0,q1,v0,k1,v1,...
    and the per-chunk projection/attention instructions are emitted in
    matching order per engine, so attention starts ~modestly after the
    first 4MB lands instead of after the full 12MB.
"""

import os
import numpy as np
import ml_dtypes

import concourse.bass as bass
import concourse.mybir as mybir
import concourse.tile as tile
from concourse import bacc
from concourse.bass import ts
from concourse.bass_utils import run_bass_kernel_spmd
from concourse.masks import make_identity

P = 128
L = 2048
D = 1024
DH = 64
NCORES = 8
KT = D // P  # 8 contraction tiles for projections
LT = L // P  # 16 seq tiles
NC = 4  # 512-col chunks per activation tensor
BF16 = mybir.dt.bfloat16
F32 = mybir.dt.float32
AF = mybir.ActivationFunctionType
ALU = mybir.AluOpType

TRACE = False  # test.py flips this to get an NTFF profile / exec_time_ns
LAST_RESULT = {}

_CACHED_NC = None


def _build():
    nc = bacc.Bacc("TRN2", target_bir_lowering=False, debug=False, num_devices=NCORES)

    qT = nc.dram_tensor("qT", [P, KT, L], BF16, kind="ExternalInput")
    kT = nc.dram_tensor("kT", [P, KT, L], BF16, kind="ExternalInput")
    vT = nc.dram_tensor("vT", [P, KT, L], BF16, kind="ExternalInput")
    wq = nc.dram_tensor("wq", [P, KT, P], BF16, kind="ExternalInput")
    wk = nc.dram_tensor("wk", [P, KT, P], BF16, kind="ExternalInput")
    wv = nc.dram_tensor("wv", [P, KT, P], BF16, kind="ExternalInput")
    bq = nc.dram_tensor("bq", [P, 1], F32, kind="ExternalInput")
    bk = nc.dram_tensor("bk", [P, 1], F32, kind="ExternalInput")
    bv = nc.dram_tensor("bv", [P, 1], F32, kind="ExternalInput")
    wo = nc.dram_tensor("wo", [P, D], BF16, kind="ExternalInput")
    bo_bc = nc.dram_tensor("bo_bc", [P, D], BF16, kind="ExternalInput")
    out = nc.dram_tensor("out", [L, D], BF16, kind="ExternalOutput")

    with tile.TileContext(nc) as tc:
        with (
            tc.tile_pool(name="const", bufs=1) as const_pool,
            tc.tile_pool(name="inputs", bufs=1) as in_pool,
            tc.tile_pool(name="proj", bufs=1) as proj_pool,
            tc.tile_pool(name="att", bufs=1) as att_pool,
            tc.tile_pool(name="pt_pool", bufs=3) as pt_pool,
            tc.tile_pool(name="ring", bufs=2) as ring_pool,
            tc.tile_pool(name="out_pool", bufs=3) as out_pool,
            tc.tile_pool(name="ps", bufs=1, space="PSUM") as ps_pool,
        ):
            # ---- staged inputs ----
            wq_sb = in_pool.tile([P, KT, P], BF16)
            wk_sb = in_pool.tile([P, KT, P], BF16)
            wv_sb = in_pool.tile([P, KT, P], BF16)
            bq_sb = in_pool.tile([P, 1], F32)
            bk_sb = in_pool.tile([P, 1], F32)
            bv_sb = in_pool.tile([P, 1], F32)
            wo_sb = in_pool.tile([P, D], BF16)
            bo_sb = in_pool.tile([P, D], BF16)
            qT_sb = in_pool.tile([P, KT, L], BF16)
            kT_sb = in_pool.tile([P, KT, L], BF16)
            vT_sb = in_pool.tile([P, KT, L], BF16)

            # Input chunks round-robin across all 16 DMA queues, so without
            # staging every chunk completes at ~the same (late) time. Chain
            # the stages on GpSimd: a tiny copy reading the previous stage's
            # last chunk delays the next stage's doorbells until that data
            # has landed, so earlier chunks get the full queue bandwidth and
            # arrive in consumption order.
            nc.sync.dma_start(wk_sb[:], wk[:])
            nc.sync.dma_start(bk_sb[:], bk[:])
            nc.sync.dma_start(wq_sb[:], wq[:])
            nc.sync.dma_start(bq_sb[:], bq[:])
            nc.sync.dma_start(wv_sb[:], wv[:])
            nc.sync.dma_start(bv_sb[:], bv[:])
            srcs = {"q": (qT_sb, qT), "k": (kT_sb, kT), "v": (vT_sb, vT)}
            stages = [
                [("k", 0), ("q", 0), ("q", 1)],
                [("v", 0), ("k", 1)],
                [("v", 1), ("k", 2)],
                [("v", 2), ("k", 3)],
                [("v", 3), ("q", 2)],
                [("q", 3), ("wo",), ("bo",)],
            ]
            dummy_sb = in_pool.tile([1, 16], BF16)
            first_gpsimd_work = [True]

            def emit_stage(si_):
                if si_ > 0:
                    t, c = [s for s in stages[si_ - 1] if len(s) == 2][-1]
                    nc.gpsimd.tensor_copy(
                        dummy_sb[0:1, 0:8], srcs[t][0][0:1, 0, c * 512 : c * 512 + 8]
                    )
                for s in stages[si_]:
                    if s == ("wo",):
                        nc.gpsimd.dma_start(wo_sb[:], wo[:])
                    elif s == ("bo",):
                        nc.gpsimd.dma_start(bo_sb[:], bo_bc[:])
                    else:
                        t, c = s
                        dst, src_ = srcs[t]
                        nc.gpsimd.dma_start(
                            dst[:, :, ts(c, 512)], src_[:, :, ts(c, 512)]
                        )

            emit_stage(0)
            # ---- constants: identity on gpsimd (before its DMA chain
            # stalls), everything else on vector which is idle early ----
            identity = const_pool.tile([P, P], BF16)
            make_identity(nc, identity[:])
            selA = const_pool.tile([1, P], BF16)
            selB = const_pool.tile([1, P], BF16)
            nc.vector.memset(selA[0:1, 0:DH], 1.0)
            nc.vector.memset(selA[0:1, DH:P], 0.0)
            nc.vector.memset(selB[0:1, 0:DH], 0.0)
            nc.vector.memset(selB[0:1, DH:P], 1.0)
            warm = const_pool.tile([1, 32], F32)
            nc.scalar.activation(warm[:], selA[0:1, 0:32], AF.Exp)

            # ---- projection outputs ----
            # khT_A: valid rows 0:64 (head A dims), rows 64:128 stay zero.
            # khT_B: valid rows 64:128, rows 0:64 stay zero.
            khT_A = proj_pool.tile([P, L], BF16)
            khT_B = proj_pool.tile([P, L], BF16)
            qhT = proj_pool.tile([P, L], BF16)
            vhT = proj_pool.tile([P, L], BF16)
            # vh per head in an M=128 stationary: col 0 = ones (so av row 0
            # accumulates the softmax denominator for free), cols 1:64 zero,
            # cols 64:128 = the head's vh. AV lands in av rows 64:128 and the
            # denominator in row 0 -- both 64-aligned partition bases.
            vh128_A = proj_pool.tile([P, LT, P], BF16)
            vh128_B = proj_pool.tile([P, LT, P], BF16)
            nc.vector.memset(khT_A[:], 0.0)
            nc.vector.memset(khT_B[:], 0.0)
            nc.vector.memset(vh128_A[:], 0.0)
            nc.vector.memset(vh128_B[:], 0.0)
            nc.vector.memset(vh128_A[:, :, 0:1], 1.0)
            nc.vector.memset(vh128_B[:, :, 0:1], 1.0)

            rfA = att_pool.tile([1, 1024], F32)
            rfB = att_pool.tile([1, 1024], F32)
            rinvA_bf = att_pool.tile([1, 1024], BF16)
            rinvB_bf = att_pool.tile([1, 1024], BF16)

            # PSUM plan (8 banks):
            #   st ring: 2 x [128, 1024] fp32 (4 banks) shared by both heads'
            #     score tiles AND (as scratch slots) projection/outproj/bc
            #     PSUM tiles -- the tag ring serializes reuse.
            #   avA/avB: [65, 1024] fp32 (2 banks each). Rows 0:64 accumulate
            #     the head's AV over all 16 kseq tiles; row 64 accumulates the
            #     softmax denominator via the ones-column in vh65 (free).
            avA = ps_pool.tile([P, 1024], F32, name="avA")
            avB = ps_pool.tile([P, 1024], F32, name="avB")

            def st_tile(name):
                return ps_pool.tile([P, 1024], F32, tag="st", bufs=2, name=name)

            def emit_proj(c, w_sb, b_sb, x_sb, kind):
                """Project 512 seq-cols (chunk c) of one input tensor."""
                ps = st_tile(f"pp_{kind}{c}")
                for t in range(KT):
                    nc.tensor.matmul(
                        ps[:, 0:512], w_sb[:, t, :], x_sb[:, t, ts(c, 512)],
                        start=(t == 0), stop=(t == KT - 1),
                    )
                if kind == "k":
                    nc.vector.tensor_scalar(
                        khT_A[0:DH, ts(c, 512)], ps[0:DH, 0:512], b_sb[0:DH],
                        None, op0=ALU.add,
                    )
                    nc.vector.tensor_scalar(
                        khT_B[DH:P, ts(c, 512)], ps[DH:P, 0:512], b_sb[DH:P],
                        None, op0=ALU.add,
                    )
                elif kind == "q":
                    nc.vector.tensor_scalar(
                        qhT[:, ts(c, 512)], ps[:, 0:512], b_sb[:], None, op0=ALU.add
                    )
                else:  # v: transpose into natural layout via the DMA xbar
                    nc.vector.tensor_scalar(
                        vhT[:, ts(c, 512)], ps[:, 0:512], b_sb[:], None, op0=ALU.add
                    )
                    for t2 in range(4 * c, 4 * c + 4):
                        nc.sync.dma_start_transpose(
                            vh128_A[:, t2, DH:P], vhT[0:DH, ts(t2, P)]
                        )
                        nc.sync.dma_start_transpose(
                            vh128_B[:, t2, DH:P], vhT[DH:P, ts(t2, P)]
                        )

            pts = {}

            def emit_scores_exp(h, kt):
                """Scores + exp for (h, kt); pt tiles parked in pts[]."""
                q0 = h * 1024
                stA = st_tile(f"stA_{h}_{kt}")
                for j in (0, 1):
                    nc.tensor.matmul(
                        stA[:, ts(j, 512)], khT_A[:, ts(kt, P)],
                        qhT[:, q0 + j * 512 : q0 + (j + 1) * 512],
                    )
                stB = st_tile(f"stB_{h}_{kt}")
                for j in (0, 1):
                    nc.tensor.matmul(
                        stB[:, ts(j, 512)], khT_B[:, ts(kt, P)],
                        qhT[:, q0 + j * 512 : q0 + (j + 1) * 512],
                    )
                ptA = pt_pool.tile([P, 1024], BF16, tag="ptA", name=f"ptA_{h}_{kt}")
                ptB = pt_pool.tile([P, 1024], BF16, tag="ptB", name=f"ptB_{h}_{kt}")
                nc.scalar.activation(ptA[:], stA[:], AF.Exp, scale=0.125)
                nc.scalar.activation(ptB[:], stB[:], AF.Exp, scale=0.125)
                pts[(h, kt)] = (ptA, ptB)

            def emit_av(h, kt):
                ptA, ptB = pts.pop((h, kt))
                for j in (0, 1):
                    nc.tensor.matmul(
                        avA[:, ts(j, 512)], vh128_A[:, kt, :], ptA[:, ts(j, 512)],
                        start=(kt == 0), stop=(kt == LT - 1),
                    )
                for j in (0, 1):
                    nc.tensor.matmul(
                        avB[:, ts(j, 512)], vh128_B[:, kt, :], ptB[:, ts(j, 512)],
                        start=(kt == 0), stop=(kt == LT - 1),
                    )

            def emit_boundary(h):
                """After last AV of half h: invert the denominators (row 64 of
                each av tile), copy the AV rows into concat^T layout."""
                u_sb = ring_pool.tile([P, 1024], BF16, tag="u", name=f"u_{h}")
                nc.vector.reciprocal_approx_fast(out=rfA[0:1, :], in_=avA[0:1, :])
                nc.vector.tensor_copy(rinvA_bf[0:1, :], rfA[0:1, :])
                nc.vector.reciprocal_approx_fast(out=rfB[0:1, :], in_=avB[0:1, :])
                nc.vector.tensor_copy(rinvB_bf[0:1, :], rfB[0:1, :])
                ueng = nc.vector if h == 0 else nc.scalar
                if h == 0:
                    ueng.tensor_copy(u_sb[0:DH, :], avA[DH:P, :])
                    ueng.tensor_copy(u_sb[DH:P, :], avB[DH:P, :])
                else:
                    nc.scalar.copy(u_sb[0:DH, :], avA[DH:P, :])
                    nc.scalar.copy(u_sb[DH:P, :], avB[DH:P, :])
                return u_sb

            def emit_bc_norm(h, u_sb):
                """Broadcast 1/d per head over its 64 partitions (selector
                matmul) and multiply into the unnormalized AV copy."""
                cT = ring_pool.tile([P, 1024], BF16, tag="cT", name=f"cT_{h}")
                bc = st_tile(f"bc_{h}")
                for j in (0, 1):
                    nc.tensor.matmul(
                        bc[:, ts(j, 512)], selA[0:1, :], rinvA_bf[0:1, ts(j, 512)],
                        start=True, stop=False,
                    )
                    nc.tensor.matmul(
                        bc[:, ts(j, 512)], selB[0:1, :], rinvB_bf[0:1, ts(j, 512)],
                        start=False, stop=True,
                    )
                nc.vector.tensor_tensor(cT[:], u_sb[:], bc[:], op=ALU.mult)
                return cT

            def emit_outproj_tile(h, m, cT, tail=False):
                mt = h * 8 + m
                osb = out_pool.tile([P, D], BF16, tag="osb", name=f"osb_{mt}")
                ops = st_tile(f"ops{mt}")
                for n in (0, 1):
                    nc.tensor.matmul(
                        ops[:, ts(n, 512)], cT[:, ts(m, P)], wo_sb[:, ts(n, 512)]
                    )
                nc.vector.tensor_tensor(osb[:], ops[:], bo_sb[:], op=ALU.add)
                nc.sync.dma_start(out[ts(mt, P), :], osb[:])

            # ---- master emission sequence ----
            emit_stage(1)
            # Warm the PE: the tensor engine clock ramps only under
            # continuous execution (0.65 -> 1.2 -> 2.4GHz after ~3us busy).
            # Dummy identity matmuls keep it spinning while the first input
            # chunks stream in, so the projections run at full rate.
            for wi in range(140):
                wp = st_tile(f"warm{wi}")
                nc.tensor.matmul(wp[:, 0:P], identity[:], identity[:])
            emit_proj(0, wk_sb, bk_sb, kT_sb, "k")
            emit_proj(0, wq_sb, bq_sb, qT_sb, "q")
            emit_proj(1, wq_sb, bq_sb, qT_sb, "q")

            # software-pipelined attention: scores/exp run one kt ahead of AV
            # so the Scalar exp stream never waits behind AV matmuls in the
            # in-order PE queue. Late projection chunks and the first half's
            # output projection are interleaved at points where their input
            # DMA has landed.
            steps = []  # (kind, args)
            for kt in range(16):
                steps.append(("att", 0, kt))
            steps.append(("boundary", 0))
            for kt in range(16):
                steps.append(("att", 1, kt))
            steps.append(("boundary", 1))
            inserts = {
                ("att", 0, 1): [("stage", 2)],
                ("att", 0, 2): [("proj", 1, "k")],
                ("att", 0, 4): [("proj", 1, "v"), ("stage", 3)],
                ("att", 0, 5): [("proj", 2, "k")],
                ("att", 0, 8): [("proj", 2, "v"), ("stage", 4)],
                ("att", 0, 9): [("proj", 3, "k")],
                ("att", 0, 12): [("proj", 3, "v"), ("stage", 5)],
                ("att", 0, 13): [("proj", 2, "q")],
                ("att", 0, 14): [("proj", 3, "q")],
                ("att", 1, 2): [("bcnorm", 0)],
            }
            outproj_at = {("att", 1, kt): kt - 3 for kt in range(3, 11)}

            emit_scores_exp(0, 0)
            emit_proj(0, wv_sb, bv_sb, vT_sb, "v")
            u_pend = {}
            cT_pend = {}
            projmap = {"k": (wk_sb, bk_sb, kT_sb), "q": (wq_sb, bq_sb, qT_sb),
                       "v": (wv_sb, bv_sb, vT_sb)}
            for si, step in enumerate(steps):
                # next scores/exp first (keeps Scalar fed), then this step's AV
                nxt = steps[si + 1] if si + 1 < len(steps) else None
                if step[0] == "att":
                    for ins in inserts.get(step, []):
                        if ins[0] == "proj":
                            w_sb, b_sb, x_sb = projmap[ins[2]]
                            emit_proj(ins[1], w_sb, b_sb, x_sb, ins[2])
                        elif ins[0] == "stage":
                            emit_stage(ins[1])
                        elif ins[0] == "bcnorm":
                            cT_pend[ins[1]] = emit_bc_norm(ins[1], u_pend[ins[1]])
                    if nxt is not None and nxt[0] == "att":
                        emit_scores_exp(nxt[1], nxt[2])
                    emit_av(step[1], step[2])
                    if step in outproj_at:
                        emit_outproj_tile(0, outproj_at[step], cT_pend[0])
                else:
                    h = step[1]
                    u_pend[h] = emit_boundary(h)
                    if nxt is not None and nxt[0] == "att":
                        emit_scores_exp(nxt[1], nxt[2])

            cT1 = emit_bc_norm(1, u_pend[1])
            for m in range(8):
                emit_outproj_tile(1, m, cT1, tail=True)

    nc.compile()
    return nc


def kernel(q, k, v, w_q, b_q, w_k, b_k, w_v, b_v, w_o, b_o):
    global _CACHED_NC, LAST_RESULT
    if _CACHED_NC is None:
        _CACHED_NC = _build()
    nc = _CACHED_NC

    bf16 = ml_dtypes.bfloat16

    def tile_T(x):  # [B, L, D] -> [128, D//128, L] contiguous
        xt = np.asarray(x, np.float32)[0].T  # [D, L]
        return np.ascontiguousarray(
            xt.reshape(D // P, P, L).transpose(1, 0, 2)
        ).astype(bf16)

    def tile_w(w):  # [D, 128] -> [128, D//128, 128] contiguous
        return np.ascontiguousarray(
            w.reshape(D // P, P, P).transpose(1, 0, 2)
        ).astype(bf16)

    q2 = tile_T(q)
    k2 = tile_T(k)
    v2 = tile_T(v)
    w_q = np.asarray(w_q, np.float32)
    w_k = np.asarray(w_k, np.float32)
    w_v = np.asarray(w_v, np.float32)
    w_o = np.asarray(w_o, np.float32)
    b_q = np.asarray(b_q, np.float32)
    b_k = np.asarray(b_k, np.float32)
    b_v = np.asarray(b_v, np.float32)
    b_o = np.asarray(b_o, np.float32)

    in_maps = []
    for i in range(NCORES):
        sl = slice(P * i, P * (i + 1))
        bo_bc_i = (
            np.ascontiguousarray(np.broadcast_to(b_o, (P, D))).astype(bf16)
            if i == 0
            else np.zeros((P, D), bf16)
        )
        in_maps.append(
            {
                "qT": q2,
                "kT": k2,
                "vT": v2,
                "wq": tile_w(w_q[:, sl]),
                "wk": tile_w(w_k[:, sl]),
                "wv": tile_w(w_v[:, sl]),
                "bq": np.ascontiguousarray(b_q[sl]).reshape(P, 1),
                "bk": np.ascontiguousarray(b_k[sl]).reshape(P, 1),
                "bv": np.ascontiguousarray(b_v[sl]).reshape(P, 1),
                "wo": np.ascontiguousarray(w_o[sl, :]).astype(bf16),
                "bo_bc": bo_bc_i,
            }
        )

    kwargs = {}
    if TRACE:
        import tempfile

        tdir = tempfile.mkdtemp(prefix="bass_trace_")
        kwargs["tmpdir"] = tdir
    res = run_bass_kernel_spmd(nc, in_maps, list(range(NCORES)), trace=TRACE, **kwargs)
    LAST_RESULT = {
        "exec_time_ns": res.exec_time_ns,
        "trace_path": (res.instructions_and_trace or (None, None))[1],
    }
    acc = np.zeros((L, D), np.float64)
    for i in range(NCORES):
        acc += res.results[i]["out"].astype(np.float64)
    return acc.astype(np.float32).reshape(1, L, D)


# revision 33
# speedup vs baseline: 1.1741x; 1.1741x over previous
"""Multi-head attention (B=1, L=2048, D=1024, H=16) on 8 TRN2 NeuronCores.

Sharding: tensor-parallel over heads. Core i computes heads 2i, 2i+1:
projections with column shards of w_q/w_k/w_v, full attention for its 2
heads, partial output projection with the matching 128-row shard of w_o.
Host sums the 8 partial outputs (row-split w_concat => partial-sum).

Redesign vs baseline (236us):
  - Uniform PE config in the hot loop: every matmul is K=128 (or tiny
    K=1/2 outside the loop), M<=128, tile_position (0,0). The baseline's
    per-head K=64/M=64 matmuls with alternating tile_position quadrants
    ran the PE at ~half rate (420-560ns per 512-row matmul vs 216ns).
    Heads are isolated by zero-padding: khT_A has head-B partition rows
    zeroed (so K=128 score matmuls don't mix heads), vh_A has head-B
    columns zeroed (so both heads' AV accumulate into one [128, L/2]
    PSUM tile without cross-talk).
  - Softmax denominators accumulated on the otherwise-idle Vector engine
    (bf16 adds of the exp tiles), column-summed with 4 small matmuls per
    half; normalize is folded into one selector matmul (K=1 pair) plus a
    DVE multiply on the unnormalized AV output.
  - exp (the Scalar-engine wall: 64 x [128,1024] tiles ~ 64us) runs on
    Scalar exclusively; all copies/bias-adds live on Vector/GpSimd.
  - b_o added on device by core 0 only, fused into the GpSimd PSUM->SBUF
    output copies (bo_bc input is zeros on other cores).
  - Input DMA split into 512-column chunks ordered k0,q0,q1,v0,k1,v1,...
    and the per-chunk projection/attention instructions are emitted in
    matching order per engine, so attention starts ~modestly after the
    first 4MB lands instead of after the full 12MB.
"""

import os
import numpy as np
import ml_dtypes

import concourse.bass as bass
import concourse.mybir as mybir
import concourse.tile as tile
from concourse import bacc
from concourse.bass import ts
from concourse.bass_utils import run_bass_kernel_spmd
from concourse.masks import make_identity

P = 128
L = 2048
D = 1024
DH = 64
NCORES = 8
KT = D // P  # 8 contraction tiles for projections
LT = L // P  # 16 seq tiles
NC = 4  # 512-col chunks per activation tensor
BF16 = mybir.dt.bfloat16
F32 = mybir.dt.float32
AF = mybir.ActivationFunctionType
ALU = mybir.AluOpType

TRACE = False  # test.py flips this to get an NTFF profile / exec_time_ns
LAST_RESULT = {}

_CACHED_NC = None


def _build():
    nc = bacc.Bacc("TRN2", target_bir_lowering=False, debug=False, num_devices=NCORES)

    qT = nc.dram_tensor("qT", [P, KT, L], BF16, kind="ExternalInput")
    kT = nc.dram_tensor("kT", [P, KT, L], BF16, kind="ExternalInput")
    vT = nc.dram_tensor("vT", [P, KT, L], BF16, kind="ExternalInput")
    wq = nc.dram_tensor("wq", [P, KT, P], BF16, kind="ExternalInput")
    wk = nc.dram_tensor("wk", [P, KT, P], BF16, kind="ExternalInput")
    wv = nc.dram_tensor("wv", [P, KT, P], BF16, kind="ExternalInput")
    bq = nc.dram_tensor("bq", [P, 1], F32, kind="ExternalInput")
    bk = nc.dram_tensor("bk", [P, 1], F32, kind="ExternalInput")
    bv = nc.dram_tensor("bv", [P, 1], F32, kind="ExternalInput")
    wo = nc.dram_tensor("wo", [P, D], BF16, kind="ExternalInput")
    out = nc.dram_tensor("out", [L, D], BF16, kind="ExternalOutput")

    with tile.TileContext(nc) as tc:
        with (
            tc.tile_pool(name="const", bufs=1) as const_pool,
            tc.tile_pool(name="inputs", bufs=1) as in_pool,
            tc.tile_pool(name="proj", bufs=1) as proj_pool,
            tc.tile_pool(name="att", bufs=1) as att_pool,
            tc.tile_pool(name="pt_pool", bufs=3) as pt_pool,
            tc.tile_pool(name="ring", bufs=2) as ring_pool,
            tc.tile_pool(name="out_pool", bufs=3) as out_pool,
            tc.tile_pool(name="ps", bufs=1, space="PSUM") as ps_pool,
        ):
            # ---- staged inputs ----
            wq_sb = in_pool.tile([P, KT, P], BF16)
            wk_sb = in_pool.tile([P, KT, P], BF16)
            wv_sb = in_pool.tile([P, KT, P], BF16)
            bq_sb = in_pool.tile([P, 1], F32)
            bk_sb = in_pool.tile([P, 1], F32)
            bv_sb = in_pool.tile([P, 1], F32)
            wo_sb = in_pool.tile([P, D], BF16)
            qT_sb = in_pool.tile([P, KT, L], BF16)
            kT_sb = in_pool.tile([P, KT, L], BF16)
            vT_sb = in_pool.tile([P, KT, L], BF16)

            # Input chunks round-robin across all 16 DMA queues, so without
            # staging every chunk completes at ~the same (late) time. Chain
            # the stages on GpSimd: a tiny copy reading the previous stage's
            # last chunk delays the next stage's doorbells until that data
            # has landed, so earlier chunks get the full queue bandwidth and
            # arrive in consumption order.
            nc.sync.dma_start(wk_sb[:], wk[:])
            nc.sync.dma_start(bk_sb[:], bk[:])
            nc.sync.dma_start(wq_sb[:], wq[:])
            nc.sync.dma_start(bq_sb[:], bq[:])
            nc.sync.dma_start(wv_sb[:], wv[:])
            nc.sync.dma_start(bv_sb[:], bv[:])
            srcs = {"q": (qT_sb, qT), "k": (kT_sb, kT), "v": (vT_sb, vT)}
            stages = [
                [("k", 0), ("q", 0), ("q", 1)],
                [("v", 0), ("k", 1)],
                [("v", 1), ("k", 2)],
                [("v", 2), ("k", 3)],
                [("v", 3), ("q", 2)],
                [("q", 3), ("wo",)],
            ]
            dummy_sb = in_pool.tile([1, 16], BF16)
            first_gpsimd_work = [True]

            def emit_stage(si_):
                if si_ > 0:
                    t, c = [s for s in stages[si_ - 1] if len(s) == 2][-1]
                    nc.gpsimd.tensor_copy(
                        dummy_sb[0:1, 0:8], srcs[t][0][0:1, 0, c * 512 : c * 512 + 8]
                    )
                for s in stages[si_]:
                    if s == ("wo",):
                        nc.gpsimd.dma_start(wo_sb[:], wo[:])
                    else:
                        t, c = s
                        dst, src_ = srcs[t]
                        nc.gpsimd.dma_start(
                            dst[:, :, ts(c, 512)], src_[:, :, ts(c, 512)]
                        )

            emit_stage(0)
            # ---- constants: identity on gpsimd (before its DMA chain
            # stalls), everything else on vector which is idle early ----
            identity = const_pool.tile([P, P], BF16)
            make_identity(nc, identity[:])
            selA = const_pool.tile([1, P], BF16)
            selB = const_pool.tile([1, P], BF16)
            nc.vector.memset(selA[0:1, 0:DH], 1.0)
            nc.vector.memset(selA[0:1, DH:P], 0.0)
            nc.vector.memset(selB[0:1, 0:DH], 0.0)
            nc.vector.memset(selB[0:1, DH:P], 1.0)
            warm = const_pool.tile([1, 32], F32)
            nc.scalar.activation(warm[:], selA[0:1, 0:32], AF.Exp)

            # ---- projection outputs ----
            # khT_A: valid rows 0:64 (head A dims), rows 64:128 stay zero.
            # khT_B: valid rows 64:128, rows 0:64 stay zero.
            khT_A = proj_pool.tile([P, L], BF16)
            khT_B = proj_pool.tile([P, L], BF16)
            qhT = proj_pool.tile([P, L], BF16)
            vhT = proj_pool.tile([P, L], BF16)
            # vh per head in an M=128 stationary: col 0 = ones (so av row 0
            # accumulates the softmax denominator for free), cols 1:64 zero,
            # cols 64:128 = the head's vh. AV lands in av rows 64:128 and the
            # denominator in row 0 -- both 64-aligned partition bases.
            vh128_A = proj_pool.tile([P, LT, P], BF16)
            vh128_B = proj_pool.tile([P, LT, P], BF16)
            nc.vector.memset(khT_A[:], 0.0)
            nc.vector.memset(khT_B[:], 0.0)
            nc.vector.memset(vh128_A[:], 0.0)
            nc.vector.memset(vh128_B[:], 0.0)
            nc.vector.memset(vh128_A[:, :, 0:1], 1.0)
            nc.vector.memset(vh128_B[:, :, 0:1], 1.0)

            rfA = att_pool.tile([1, 1024], F32)
            rfB = att_pool.tile([1, 1024], F32)
            rinvA_bf = att_pool.tile([1, 1024], BF16)
            rinvB_bf = att_pool.tile([1, 1024], BF16)

            # PSUM plan (8 banks):
            #   st ring: 2 x [128, 1024] fp32 (4 banks) shared by both heads'
            #     score tiles AND (as scratch slots) projection/outproj/bc
            #     PSUM tiles -- the tag ring serializes reuse.
            #   avA/avB: [65, 1024] fp32 (2 banks each). Rows 0:64 accumulate
            #     the head's AV over all 16 kseq tiles; row 64 accumulates the
            #     softmax denominator via the ones-column in vh65 (free).
            avA = ps_pool.tile([P, 1024], F32, name="avA")
            avB = ps_pool.tile([P, 1024], F32, name="avB")

            def st_tile(name):
                return ps_pool.tile([P, 1024], F32, tag="st", bufs=2, name=name)

            def emit_proj(c, w_sb, b_sb, x_sb, kind):
                """Project 512 seq-cols (chunk c) of one input tensor."""
                ps = st_tile(f"pp_{kind}{c}")
                for t in range(KT):
                    nc.tensor.matmul(
                        ps[:, 0:512], w_sb[:, t, :], x_sb[:, t, ts(c, 512)],
                        start=(t == 0), stop=(t == KT - 1),
                    )
                if kind == "k":
                    nc.vector.tensor_scalar(
                        khT_A[0:DH, ts(c, 512)], ps[0:DH, 0:512], b_sb[0:DH],
                        None, op0=ALU.add,
                    )
                    nc.vector.tensor_scalar(
                        khT_B[DH:P, ts(c, 512)], ps[DH:P, 0:512], b_sb[DH:P],
                        None, op0=ALU.add,
                    )
                elif kind == "q":
                    nc.vector.tensor_scalar(
                        qhT[:, ts(c, 512)], ps[:, 0:512], b_sb[:], None, op0=ALU.add
                    )
                else:  # v: also transpose 4 seq-tiles into natural layout
                    nc.vector.tensor_scalar(
                        vhT[:, ts(c, 512)], ps[:, 0:512], b_sb[:], None, op0=ALU.add
                    )
                    for t2 in range(4 * c, 4 * c + 4):
                        pst = ps_pool.tile(
                            [P, P], BF16, tag="st", bufs=2, name=f"pst{t2}"
                        )
                        nc.tensor.transpose(pst[:], vhT[:, ts(t2, P)], identity[:])
                        nc.vector.tensor_copy(vh128_A[:, t2, DH:P], pst[:, 0:DH])
                        nc.vector.tensor_copy(vh128_B[:, t2, DH:P], pst[:, DH:P])

            pts = {}

            def emit_scores_exp(h, kt):
                """Scores + exp for (h, kt); pt tiles parked in pts[]."""
                q0 = h * 1024
                stA = st_tile(f"stA_{h}_{kt}")
                for j in (0, 1):
                    nc.tensor.matmul(
                        stA[:, ts(j, 512)], khT_A[:, ts(kt, P)],
                        qhT[:, q0 + j * 512 : q0 + (j + 1) * 512],
                    )
                stB = st_tile(f"stB_{h}_{kt}")
                for j in (0, 1):
                    nc.tensor.matmul(
                        stB[:, ts(j, 512)], khT_B[:, ts(kt, P)],
                        qhT[:, q0 + j * 512 : q0 + (j + 1) * 512],
                    )
                ptA = pt_pool.tile([P, 1024], BF16, tag="ptA", name=f"ptA_{h}_{kt}")
                ptB = pt_pool.tile([P, 1024], BF16, tag="ptB", name=f"ptB_{h}_{kt}")
                nc.scalar.activation(ptA[:], stA[:], AF.Exp, scale=0.125)
                nc.scalar.activation(ptB[:], stB[:], AF.Exp, scale=0.125)
                pts[(h, kt)] = (ptA, ptB)

            def emit_av(h, kt):
                ptA, ptB = pts.pop((h, kt))
                for j in (0, 1):
                    nc.tensor.matmul(
                        avA[:, ts(j, 512)], vh128_A[:, kt, :], ptA[:, ts(j, 512)],
                        start=(kt == 0), stop=(kt == LT - 1),
                    )
                for j in (0, 1):
                    nc.tensor.matmul(
                        avB[:, ts(j, 512)], vh128_B[:, kt, :], ptB[:, ts(j, 512)],
                        start=(kt == 0), stop=(kt == LT - 1),
                    )

            def emit_boundary(h):
                """After last AV of half h: invert the denominators (row 64 of
                each av tile), copy the AV rows into concat^T layout."""
                u_sb = ring_pool.tile([P, 1024], BF16, tag="u", name=f"u_{h}")
                nc.vector.reciprocal_approx_fast(out=rfA[0:1, :], in_=avA[0:1, :])
                nc.vector.tensor_copy(rinvA_bf[0:1, :], rfA[0:1, :])
                nc.vector.reciprocal_approx_fast(out=rfB[0:1, :], in_=avB[0:1, :])
                nc.vector.tensor_copy(rinvB_bf[0:1, :], rfB[0:1, :])
                ueng = nc.vector if h == 0 else nc.scalar
                if h == 0:
                    ueng.tensor_copy(u_sb[0:DH, :], avA[DH:P, :])
                    ueng.tensor_copy(u_sb[DH:P, :], avB[DH:P, :])
                else:
                    nc.scalar.copy(u_sb[0:DH, :], avA[DH:P, :])
                    nc.scalar.copy(u_sb[DH:P, :], avB[DH:P, :])
                return u_sb

            def emit_bc_norm(h, u_sb):
                """Broadcast 1/d per head over its 64 partitions (selector
                matmul) and multiply into the unnormalized AV copy."""
                cT = ring_pool.tile([P, 1024], BF16, tag="cT", name=f"cT_{h}")
                bc = st_tile(f"bc_{h}")
                for j in (0, 1):
                    nc.tensor.matmul(
                        bc[:, ts(j, 512)], selA[0:1, :], rinvA_bf[0:1, ts(j, 512)],
                        start=True, stop=False,
                    )
                    nc.tensor.matmul(
                        bc[:, ts(j, 512)], selB[0:1, :], rinvB_bf[0:1, ts(j, 512)],
                        start=False, stop=True,
                    )
                nc.vector.tensor_tensor(cT[:], u_sb[:], bc[:], op=ALU.mult)
                return cT

            def emit_outproj_tile(h, m, cT, tail=False):
                mt = h * 8 + m
                osb = out_pool.tile([P, D], BF16, tag="osb", name=f"osb_{mt}")
                ops = st_tile(f"ops{mt}")
                for n in (0, 1):
                    nc.tensor.matmul(
                        ops[:, ts(n, 512)], cT[:, ts(m, P)], wo_sb[:, ts(n, 512)]
                    )
                if tail and m % 2 == 0:
                    nc.scalar.copy(osb[:], ops[:])
                else:
                    nc.vector.tensor_copy(osb[:], ops[:])
                nc.sync.dma_start(out[ts(mt, P), :], osb[:])

            # ---- master emission sequence ----
            emit_stage(1)
            # Warm the PE: the tensor engine clock ramps only under
            # continuous execution (0.65 -> 1.2 -> 2.4GHz after ~3us busy).
            # Dummy identity matmuls into avA (overwritten by the first real
            # AV accumulation's start=True) keep it spinning while the first
            # input chunks stream in, so the projections run at full rate.
            for wi in range(90):
                nc.tensor.matmul(avA[:, 0:P], identity[:], identity[:])
            emit_proj(0, wk_sb, bk_sb, kT_sb, "k")
            emit_proj(0, wq_sb, bq_sb, qT_sb, "q")
            emit_proj(1, wq_sb, bq_sb, qT_sb, "q")

            # software-pipelined attention: scores/exp run one kt ahead of AV
            # so the Scalar exp stream never waits behind AV matmuls in the
            # in-order PE queue. Late projection chunks and the first half's
            # output projection are interleaved at points where their input
            # DMA has landed.
            steps = []  # (kind, args)
            for kt in range(16):
                steps.append(("att", 0, kt))
            steps.append(("boundary", 0))
            for kt in range(16):
                steps.append(("att", 1, kt))
            steps.append(("boundary", 1))
            inserts = {
                ("att", 0, 1): [("stage", 2)],
                ("att", 0, 2): [("proj", 1, "k")],
                ("att", 0, 4): [("proj", 1, "v"), ("stage", 3)],
                ("att", 0, 5): [("proj", 2, "k")],
                ("att", 0, 8): [("proj", 2, "v"), ("stage", 4)],
                ("att", 0, 9): [("proj", 3, "k")],
                ("att", 0, 12): [("proj", 3, "v"), ("stage", 5)],
                ("att", 0, 13): [("proj", 2, "q")],
                ("att", 0, 14): [("proj", 3, "q")],
                ("att", 1, 2): [("bcnorm", 0)],
            }
            outproj_at = {("att", 1, kt): kt - 3 for kt in range(3, 11)}

            emit_scores_exp(0, 0)
            emit_proj(0, wv_sb, bv_sb, vT_sb, "v")
            u_pend = {}
            cT_pend = {}
            projmap = {"k": (wk_sb, bk_sb, kT_sb), "q": (wq_sb, bq_sb, qT_sb),
                       "v": (wv_sb, bv_sb, vT_sb)}
            for si, step in enumerate(steps):
                # next scores/exp first (keeps Scalar fed), then this step's AV
                nxt = steps[si + 1] if si + 1 < len(steps) else None
                if step[0] == "att":
                    for ins in inserts.get(step, []):
                        if ins[0] == "proj":
                            w_sb, b_sb, x_sb = projmap[ins[2]]
                            emit_proj(ins[1], w_sb, b_sb, x_sb, ins[2])
                        elif ins[0] == "stage":
                            emit_stage(ins[1])
                        elif ins[0] == "bcnorm":
                            cT_pend[ins[1]] = emit_bc_norm(ins[1], u_pend[ins[1]])
                    if nxt is not None and nxt[0] == "att":
                        emit_scores_exp(nxt[1], nxt[2])
                    emit_av(step[1], step[2])
                    if step in outproj_at:
                        emit_outproj_tile(0, outproj_at[step], cT_pend[0])
                else:
                    h = step[1]
                    u_pend[h] = emit_boundary(h)
                    if nxt is not None and nxt[0] == "att":
                        emit_scores_exp(nxt[1], nxt[2])

            cT1 = emit_bc_norm(1, u_pend[1])
            for m in range(8):
                emit_outproj_tile(1, m, cT1, tail=True)

    nc.compile()
    return nc


def kernel(q, k, v, w_q, b_q, w_k, b_k, w_v, b_v, w_o, b_o):
    global _CACHED_NC, LAST_RESULT
    if _CACHED_NC is None:
        _CACHED_NC = _build()
    nc = _CACHED_NC

    bf16 = ml_dtypes.bfloat16

    def tile_T(x):  # [B, L, D] -> [128, D//128, L] contiguous
        xt = np.asarray(x, np.float32)[0].T  # [D, L]
        return np.ascontiguousarray(
            xt.reshape(D // P, P, L).transpose(1, 0, 2)
        ).astype(bf16)

    def tile_w(w):  # [D, 128] -> [128, D//128, 128] contiguous
        return np.ascontiguousarray(
            w.reshape(D // P, P, P).transpose(1, 0, 2)
        ).astype(bf16)

    q2 = tile_T(q)
    k2 = tile_T(k)
    v2 = tile_T(v)
    w_q = np.asarray(w_q, np.float32)
    w_k = np.asarray(w_k, np.float32)
    w_v = np.asarray(w_v, np.float32)
    w_o = np.asarray(w_o, np.float32)
    b_q = np.asarray(b_q, np.float32)
    b_k = np.asarray(b_k, np.float32)
    b_v = np.asarray(b_v, np.float32)
    b_o = np.asarray(b_o, np.float32)

    in_maps = []
    for i in range(NCORES):
        sl = slice(P * i, P * (i + 1))
        in_maps.append(
            {
                "qT": q2,
                "kT": k2,
                "vT": v2,
                "wq": tile_w(w_q[:, sl]),
                "wk": tile_w(w_k[:, sl]),
                "wv": tile_w(w_v[:, sl]),
                "bq": np.ascontiguousarray(b_q[sl]).reshape(P, 1),
                "bk": np.ascontiguousarray(b_k[sl]).reshape(P, 1),
                "bv": np.ascontiguousarray(b_v[sl]).reshape(P, 1),
                "wo": np.ascontiguousarray(w_o[sl, :]).astype(bf16),
            }
        )

    kwargs = {}
    if TRACE:
        import tempfile

        tdir = tempfile.mkdtemp(prefix="bass_trace_")
        kwargs["tmpdir"] = tdir
    res = run_bass_kernel_spmd(nc, in_maps, list(range(NCORES)), trace=TRACE, **kwargs)
    LAST_RESULT = {
        "exec_time_ns": res.exec_time_ns,
        "trace_path": (res.instructions_and_trace or (None, None))[1],
    }
    acc = np.zeros((L, D), np.float64)
    for i in range(NCORES):
        acc += res.results[i]["out"].astype(np.float64)
    acc += b_o.astype(np.float64)
    return acc.astype(np.float32).reshape(1, L, D)


# revision 41
# speedup vs baseline: 1.1819x; 1.0066x over previous
"""Multi-head attention (B=1, L=2048, D=1024, H=16) on 8 TRN2 NeuronCores.

Sharding: tensor-parallel over heads. Core i computes heads 2i, 2i+1:
projections with column shards of w_q/w_k/w_v, full attention for its 2
heads, partial output projection with the matching 128-row shard of w_o.
Host sums the 8 partial outputs and adds b_o (row-split w_concat =>
partial-sum combine).

Design (236us baseline -> ~154us):
  - Uniform PE config in the hot loop: every matmul is K=128, M=128,
    tile_position (0,0). The baseline's K=64/M=64 matmuls with
    alternating tile_position quadrants ran the PE at half rate
    (420-560ns per 512-row matmul vs the 216ns these reach). Heads are
    isolated by zero padding: khT_A has head-B partition rows zeroed so
    K=128 score matmuls don't mix heads.
  - vh per head is a [128(kseq), 128] stationary with col 0 = ones,
    cols 1:64 = 0, cols 64:128 = vh. Each head's AV accumulates into
    its own [128, 1024] PSUM tile: rows 64:128 collect the AV, row 0
    collects the softmax DENOMINATOR for free (the ones column), so no
    separate reduction is needed. reciprocal_approx_fast inverts the
    denominator row directly (base partition 0; the custom DVE op
    breaks at non-zero partition bases on HW).
  - Normalization: 1/d broadcast over each head's 64 partitions with a
    K=1 selector-matmul pair, one DVE multiply on the bf16 copy of av.
    Concat^T layout is assembled with partition-shifted DVE copies
    (HW-verified legal for 64-aligned bases).
  - exp is the Scalar-engine wall (64 x [128,1024] tiles ~ 64us); the
    emission is software-pipelined so scores/exp run one kseq-tile
    ahead of the AV matmuls and the exp stream stays fed. Everything
    else (casts, copies) lives on Vector/GpSimd.
  - Input DMA is issued in dependency-chained stages on GpSimd (a tiny
    copy reading the previous stage's last chunk gates the next stage's
    doorbells): without this, all chunks round-robin across the 16 DMA
    queues and everything lands equally late. Late-needed chunks (k1-3,
    v2-3, q2-3) are projected by instructions interleaved into the
    attention stream at points where their data has arrived.
  - Dummy identity matmuls (a live accumulation chain so they can't be
    dead-code eliminated) warm the PE clock during the initial DMA and
    across the final boundary (the PE p-state drops to 1.2GHz after any
    idle gap and needs ~3us of continuous work to reach 2.4GHz).
"""

import os
import numpy as np
import ml_dtypes

import concourse.bass as bass
import concourse.mybir as mybir
import concourse.tile as tile
from concourse import bacc
from concourse.bass import ts
from concourse.bass_utils import run_bass_kernel_spmd
from concourse.masks import make_identity

P = 128
L = 2048
D = 1024
DH = 64
NCORES = 8
KT = D // P  # 8 contraction tiles for projections
LT = L // P  # 16 seq tiles
NC = 4  # 512-col chunks per activation tensor
BF16 = mybir.dt.bfloat16
F32 = mybir.dt.float32
AF = mybir.ActivationFunctionType
ALU = mybir.AluOpType

TRACE = False  # test.py flips this to get an NTFF profile / exec_time_ns
LAST_RESULT = {}

_CACHED_NC = None


def _build():
    nc = bacc.Bacc("TRN2", target_bir_lowering=False, debug=False, num_devices=NCORES)

    qT = nc.dram_tensor("qT", [P, KT, L], BF16, kind="ExternalInput")
    kT = nc.dram_tensor("kT", [P, KT, L], BF16, kind="ExternalInput")
    vT = nc.dram_tensor("vT", [P, KT, L], BF16, kind="ExternalInput")
    wq = nc.dram_tensor("wq", [P, KT, P], BF16, kind="ExternalInput")
    wk = nc.dram_tensor("wk", [P, KT, P], BF16, kind="ExternalInput")
    wv = nc.dram_tensor("wv", [P, KT, P], BF16, kind="ExternalInput")
    bq = nc.dram_tensor("bq", [P, 1], F32, kind="ExternalInput")
    bk = nc.dram_tensor("bk", [P, 1], F32, kind="ExternalInput")
    bv = nc.dram_tensor("bv", [P, 1], F32, kind="ExternalInput")
    wo = nc.dram_tensor("wo", [P, D], BF16, kind="ExternalInput")
    out = nc.dram_tensor("out", [L, D], BF16, kind="ExternalOutput")

    with tile.TileContext(nc) as tc:
        with (
            tc.tile_pool(name="const", bufs=1) as const_pool,
            tc.tile_pool(name="inputs", bufs=1) as in_pool,
            tc.tile_pool(name="proj", bufs=1) as proj_pool,
            tc.tile_pool(name="att", bufs=1) as att_pool,
            tc.tile_pool(name="pt_pool", bufs=3) as pt_pool,
            tc.tile_pool(name="ring", bufs=2) as ring_pool,
            tc.tile_pool(name="out_pool", bufs=3) as out_pool,
            tc.tile_pool(name="ps", bufs=1, space="PSUM") as ps_pool,
        ):
            # ---- staged inputs ----
            wq_sb = in_pool.tile([P, KT, P], BF16)
            wk_sb = in_pool.tile([P, KT, P], BF16)
            wv_sb = in_pool.tile([P, KT, P], BF16)
            bq_sb = in_pool.tile([P, 1], F32)
            bk_sb = in_pool.tile([P, 1], F32)
            bv_sb = in_pool.tile([P, 1], F32)
            wo_sb = in_pool.tile([P, D], BF16)
            qT_sb = in_pool.tile([P, KT, L], BF16)
            kT_sb = in_pool.tile([P, KT, L], BF16)
            vT_sb = in_pool.tile([P, KT, L], BF16)

            # Input chunks round-robin across all 16 DMA queues, so without
            # staging every chunk completes at ~the same (late) time. Chain
            # the stages on GpSimd: a tiny copy reading the previous stage's
            # last chunk delays the next stage's doorbells until that data
            # has landed, so earlier chunks get the full queue bandwidth and
            # arrive in consumption order.
            nc.sync.dma_start(wk_sb[:], wk[:])
            nc.sync.dma_start(bk_sb[:], bk[:])
            nc.sync.dma_start(wq_sb[:], wq[:])
            nc.sync.dma_start(bq_sb[:], bq[:])
            nc.sync.dma_start(wv_sb[:], wv[:])
            nc.sync.dma_start(bv_sb[:], bv[:])
            srcs = {"q": (qT_sb, qT), "k": (kT_sb, kT), "v": (vT_sb, vT)}
            stages = [
                [("k", 0)],
                [("q", 0), ("q", 1), ("v", 0), ("v", 1)],
                [("k", 1)],
                [("v", 2), ("k", 2)],
                [("v", 3), ("k", 3)],
                [("q", 2), ("q", 3), ("wo",)],
            ]
            dummy_sb = in_pool.tile([1, 16], BF16)
            first_gpsimd_work = [True]

            def emit_stage(si_):
                if si_ > 0:
                    t, c = [s for s in stages[si_ - 1] if len(s) == 2][-1]
                    nc.gpsimd.tensor_copy(
                        dummy_sb[0:1, 0:8], srcs[t][0][0:1, 0, c * 512 : c * 512 + 8]
                    )
                for s in stages[si_]:
                    if s == ("wo",):
                        nc.gpsimd.dma_start(wo_sb[:], wo[:])
                    else:
                        t, c = s
                        dst, src_ = srcs[t]
                        nc.gpsimd.dma_start(
                            dst[:, :, ts(c, 512)], src_[:, :, ts(c, 512)]
                        )

            emit_stage(0)
            # ---- constants: identity on gpsimd (before its DMA chain
            # stalls), everything else on vector which is idle early ----
            identity = const_pool.tile([P, P], BF16)
            make_identity(nc, identity[:])
            selA = const_pool.tile([1, P], BF16)
            selB = const_pool.tile([1, P], BF16)
            nc.vector.memset(selA[0:1, 0:DH], 1.0)
            nc.vector.memset(selA[0:1, DH:P], 0.0)
            nc.vector.memset(selB[0:1, 0:DH], 0.0)
            nc.vector.memset(selB[0:1, DH:P], 1.0)
            warm = const_pool.tile([1, 32], F32)
            nc.scalar.activation(warm[:], selA[0:1, 0:32], AF.Exp)

            # ---- projection outputs ----
            # khT_A: valid rows 0:64 (head A dims), rows 64:128 stay zero.
            # khT_B: valid rows 64:128, rows 0:64 stay zero.
            khT_A = proj_pool.tile([P, L], BF16)
            khT_B = proj_pool.tile([P, L], BF16)
            qhT = proj_pool.tile([P, L], BF16)
            vhT = proj_pool.tile([P, L], BF16)
            # vh per head in an M=128 stationary: col 0 = ones (so av row 0
            # accumulates the softmax denominator for free), cols 1:64 zero,
            # cols 64:128 = the head's vh. AV lands in av rows 64:128 and the
            # denominator in row 0 -- both 64-aligned partition bases.
            vh128_A = proj_pool.tile([P, LT, P], BF16)
            vh128_B = proj_pool.tile([P, LT, P], BF16)
            nc.vector.memset(khT_A[:], 0.0)
            nc.vector.memset(khT_B[:], 0.0)
            nc.vector.memset(vh128_A[:], 0.0)
            nc.vector.memset(vh128_B[:], 0.0)
            nc.vector.memset(vh128_A[:, :, 0:1], 1.0)
            nc.vector.memset(vh128_B[:, :, 0:1], 1.0)

            rfA = att_pool.tile([1, 1024], F32)
            rfB = att_pool.tile([1, 1024], F32)
            rinvA_bf = att_pool.tile([1, 1024], BF16)
            rinvB_bf = att_pool.tile([1, 1024], BF16)

            # PSUM plan (8 banks):
            #   st ring: 2 x [128, 1024] fp32 (4 banks) shared by both heads'
            #     score tiles AND (as scratch slots) projection/outproj/bc
            #     PSUM tiles -- the tag ring serializes reuse.
            #   avA/avB: [65, 1024] fp32 (2 banks each). Rows 0:64 accumulate
            #     the head's AV over all 16 kseq tiles; row 64 accumulates the
            #     softmax denominator via the ones-column in vh65 (free).
            avA = ps_pool.tile([P, 1024], F32, name="avA")
            avB = ps_pool.tile([P, 1024], F32, name="avB")

            def st_tile(name):
                return ps_pool.tile([P, 1024], F32, tag="st", bufs=2, name=name)

            def emit_proj(c, w_sb, b_sb, x_sb, kind):
                """Project 512 seq-cols (chunk c) of one input tensor."""
                ps = st_tile(f"pp_{kind}{c}")
                for t in range(KT):
                    nc.tensor.matmul(
                        ps[:, 0:512], w_sb[:, t, :], x_sb[:, t, ts(c, 512)],
                        start=(t == 0), stop=(t == KT - 1),
                    )
                if kind == "k":
                    nc.vector.tensor_scalar(
                        khT_A[0:DH, ts(c, 512)], ps[0:DH, 0:512], b_sb[0:DH],
                        None, op0=ALU.add,
                    )
                    nc.vector.tensor_scalar(
                        khT_B[DH:P, ts(c, 512)], ps[DH:P, 0:512], b_sb[DH:P],
                        None, op0=ALU.add,
                    )
                elif kind == "q":
                    nc.vector.tensor_scalar(
                        qhT[:, ts(c, 512)], ps[:, 0:512], b_sb[:], None, op0=ALU.add
                    )
                else:  # v: also transpose 4 seq-tiles into natural layout
                    nc.vector.tensor_scalar(
                        vhT[:, ts(c, 512)], ps[:, 0:512], b_sb[:], None, op0=ALU.add
                    )
                    for t2 in range(4 * c, 4 * c + 4):
                        pst = ps_pool.tile(
                            [P, P], BF16, tag="st", bufs=2, name=f"pst{t2}"
                        )
                        nc.tensor.transpose(pst[:], vhT[:, ts(t2, P)], identity[:])
                        nc.vector.tensor_copy(vh128_A[:, t2, DH:P], pst[:, 0:DH])
                        nc.vector.tensor_copy(vh128_B[:, t2, DH:P], pst[:, DH:P])

            pts = {}

            def emit_scores_exp(h, kt):
                """Scores + exp for (h, kt); pt tiles parked in pts[]."""
                q0 = h * 1024
                stA = st_tile(f"stA_{h}_{kt}")
                for j in (0, 1):
                    nc.tensor.matmul(
                        stA[:, ts(j, 512)], khT_A[:, ts(kt, P)],
                        qhT[:, q0 + j * 512 : q0 + (j + 1) * 512],
                    )
                stB = st_tile(f"stB_{h}_{kt}")
                for j in (0, 1):
                    nc.tensor.matmul(
                        stB[:, ts(j, 512)], khT_B[:, ts(kt, P)],
                        qhT[:, q0 + j * 512 : q0 + (j + 1) * 512],
                    )
                ptA = pt_pool.tile([P, 1024], BF16, tag="ptA", name=f"ptA_{h}_{kt}")
                ptB = pt_pool.tile([P, 1024], BF16, tag="ptB", name=f"ptB_{h}_{kt}")
                nc.scalar.activation(ptA[:], stA[:], AF.Exp, scale=0.125)
                nc.scalar.activation(ptB[:], stB[:], AF.Exp, scale=0.125)
                pts[(h, kt)] = (ptA, ptB)

            def emit_av(h, kt):
                ptA, ptB = pts.pop((h, kt))
                for j in (0, 1):
                    nc.tensor.matmul(
                        avA[:, ts(j, 512)], vh128_A[:, kt, :], ptA[:, ts(j, 512)],
                        start=(kt == 0), stop=(kt == LT - 1),
                    )
                for j in (0, 1):
                    nc.tensor.matmul(
                        avB[:, ts(j, 512)], vh128_B[:, kt, :], ptB[:, ts(j, 512)],
                        start=(kt == 0), stop=(kt == LT - 1),
                    )

            def emit_boundary(h):
                """After last AV of half h: invert the denominators (row 64 of
                each av tile), copy the AV rows into concat^T layout."""
                u_sb = ring_pool.tile([P, 1024], BF16, tag="u", name=f"u_{h}")
                nc.vector.reciprocal_approx_fast(out=rfA[0:1, :], in_=avA[0:1, :])
                nc.vector.reciprocal_approx_fast(out=rfB[0:1, :], in_=avB[0:1, :])
                nc.vector.tensor_copy(rinvA_bf[0:1, :], rfA[0:1, :])
                nc.vector.tensor_copy(rinvB_bf[0:1, :], rfB[0:1, :])
                ueng = nc.vector if h == 0 else nc.scalar
                if h == 0:
                    ueng.tensor_copy(u_sb[0:DH, :], avA[DH:P, :])
                    ueng.tensor_copy(u_sb[DH:P, :], avB[DH:P, :])
                else:
                    nc.scalar.copy(u_sb[0:DH, :], avA[DH:P, :])
                    nc.scalar.copy(u_sb[DH:P, :], avB[DH:P, :])
                return u_sb

            def emit_bc_norm(h, u_sb):
                """Broadcast 1/d per head over its 64 partitions (selector
                matmul) and multiply into the unnormalized AV copy."""
                cT = ring_pool.tile([P, 1024], BF16, tag="cT", name=f"cT_{h}")
                bc = st_tile(f"bc_{h}")
                for j in (0, 1):
                    nc.tensor.matmul(
                        bc[:, ts(j, 512)], selA[0:1, :], rinvA_bf[0:1, ts(j, 512)],
                        start=True, stop=False,
                    )
                    nc.tensor.matmul(
                        bc[:, ts(j, 512)], selB[0:1, :], rinvB_bf[0:1, ts(j, 512)],
                        start=False, stop=True,
                    )
                nc.vector.tensor_tensor(cT[:], u_sb[:], bc[:], op=ALU.mult)
                return cT

            def emit_outproj_tile(h, m, cT, tail=False):
                mt = h * 8 + m
                osb = out_pool.tile([P, D], BF16, tag="osb", name=f"osb_{mt}")
                ops = st_tile(f"ops{mt}")
                for n in (0, 1):
                    nc.tensor.matmul(
                        ops[:, ts(n, 512)], cT[:, ts(m, P)], wo_sb[:, ts(n, 512)]
                    )
                if tail:
                    nc.scalar.copy(osb[:, 0:512], ops[:, 0:512])
                    nc.vector.tensor_copy(osb[:, 512:D], ops[:, 512:D])
                    nc.sync.dma_start(out[ts(mt, P), 0:512], osb[:, 0:512])
                    nc.sync.dma_start(out[ts(mt, P), 512:D], osb[:, 512:D])
                else:
                    nc.vector.tensor_copy(osb[:], ops[:])
                    nc.sync.dma_start(out[ts(mt, P), :], osb[:])

            # ---- master emission sequence ----
            # Warm the PE: the tensor engine clock ramps only under
            # continuous execution (0.65 -> 1.2 -> 2.4GHz after ~3us busy).
            # Dummy identity matmuls (one live accumulation chain with a
            # reader, so they can't be dead-code eliminated; the first real
            # AV matmul's start=True later resets the region) keep it
            # spinning while the first input chunks stream in, so the
            # projections run at full rate.
            def emit_warm(n, tag):
                wp = st_tile(tag)
                for wi in range(n):
                    nc.tensor.matmul(
                        wp[:, 0:P], identity[:], identity[:],
                        start=(wi == 0), stop=(wi == n - 1),
                    )
                nc.vector.tensor_copy(dummy_sb[0:1, 8:16], wp[0:1, 0:8])
            emit_stage(1)
            emit_warm(90, "w0")
            emit_proj(0, wk_sb, bk_sb, kT_sb, "k")
            emit_proj(0, wq_sb, bq_sb, qT_sb, "q")
            emit_proj(1, wq_sb, bq_sb, qT_sb, "q")

            # software-pipelined attention: scores/exp run one kt ahead of AV
            # so the Scalar exp stream never waits behind AV matmuls in the
            # in-order PE queue. Late projection chunks and the first half's
            # output projection are interleaved at points where their input
            # DMA has landed.
            steps = []  # (kind, args)
            for kt in range(16):
                steps.append(("att", 0, kt))
            steps.append(("boundary", 0))
            for kt in range(16):
                steps.append(("att", 1, kt))
            steps.append(("boundary", 1))
            inserts = {
                ("att", 0, 0): [("stage", 2)],
                ("att", 0, 2): [("stage", 3)],
                ("att", 0, 3): [("proj", 1, "k")],
                ("att", 0, 5): [("proj", 2, "v")],
                ("att", 0, 6): [("stage", 4)],
                ("att", 0, 7): [("proj", 2, "k")],
                ("att", 0, 9): [("proj", 3, "v")],
                ("att", 0, 10): [("stage", 5)],
                ("att", 0, 11): [("proj", 3, "k")],
                ("att", 0, 13): [("proj", 2, "q")],
                ("att", 0, 14): [("proj", 3, "q")],
                ("att", 1, 2): [("bcnorm", 0)],
            }
            outproj_at = {("att", 1, kt): kt - 3 for kt in range(3, 11)}

            emit_scores_exp(0, 0)
            emit_proj(0, wv_sb, bv_sb, vT_sb, "v")
            emit_scores_exp(0, 1)
            emit_proj(1, wv_sb, bv_sb, vT_sb, "v")
            pre_emitted = {("att", 0, 1)}
            u_pend = {}
            cT_pend = {}
            projmap = {"k": (wk_sb, bk_sb, kT_sb), "q": (wq_sb, bq_sb, qT_sb),
                       "v": (wv_sb, bv_sb, vT_sb)}
            for si, step in enumerate(steps):
                # next scores/exp first (keeps Scalar fed), then this step's AV
                nxt = steps[si + 1] if si + 1 < len(steps) else None
                if step[0] == "att":
                    for ins in inserts.get(step, []):
                        if ins[0] == "proj":
                            w_sb, b_sb, x_sb = projmap[ins[2]]
                            emit_proj(ins[1], w_sb, b_sb, x_sb, ins[2])
                        elif ins[0] == "stage":
                            emit_stage(ins[1])

                        elif ins[0] == "bcnorm":
                            cT_pend[ins[1]] = emit_bc_norm(ins[1], u_pend[ins[1]])
                    if nxt is not None and nxt[0] == "att" and nxt not in pre_emitted:
                        emit_scores_exp(nxt[1], nxt[2])
                    emit_av(step[1], step[2])
                    if step in outproj_at:
                        emit_outproj_tile(0, outproj_at[step], cT_pend[0])
                else:
                    h = step[1]
                    u_pend[h] = emit_boundary(h)
                    if nxt is not None and nxt[0] == "att":
                        emit_scores_exp(nxt[1], nxt[2])

            emit_warm(60, "w1")
            cT1 = emit_bc_norm(1, u_pend[1])
            for m in range(8):
                emit_outproj_tile(1, m, cT1, tail=True)

    nc.compile()
    return nc


def kernel(q, k, v, w_q, b_q, w_k, b_k, w_v, b_v, w_o, b_o):
    global _CACHED_NC, LAST_RESULT
    if _CACHED_NC is None:
        _CACHED_NC = _build()
    nc = _CACHED_NC

    bf16 = ml_dtypes.bfloat16

    def tile_T(x):  # [B, L, D] -> [128, D//128, L] contiguous
        xt = np.asarray(x, np.float32)[0].T  # [D, L]
        return np.ascontiguousarray(
            xt.reshape(D // P, P, L).transpose(1, 0, 2)
        ).astype(bf16)

    def tile_w(w):  # [D, 128] -> [128, D//128, 128] contiguous
        return np.ascontiguousarray(
            w.reshape(D // P, P, P).transpose(1, 0, 2)
        ).astype(bf16)

    q2 = tile_T(q)
    k2 = tile_T(k)
    v2 = tile_T(v)
    w_q = np.asarray(w_q, np.float32)
    w_k = np.asarray(w_k, np.float32)
    w_v = np.asarray(w_v, np.float32)
    w_o = np.asarray(w_o, np.float32)
    b_q = np.asarray(b_q, np.float32)
    b_k = np.asarray(b_k, np.float32)
    b_v = np.asarray(b_v, np.float32)
    b_o = np.asarray(b_o, np.float32)

    in_maps = []
    for i in range(NCORES):
        sl = slice(P * i, P * (i + 1))
        in_maps.append(
            {
                "qT": q2,
                "kT": k2,
                "vT": v2,
                "wq": tile_w(w_q[:, sl]),
                "wk": tile_w(w_k[:, sl]),
                "wv": tile_w(w_v[:, sl]),
                "bq": np.ascontiguousarray(b_q[sl]).reshape(P, 1),
                "bk": np.ascontiguousarray(b_k[sl]).reshape(P, 1),
                "bv": np.ascontiguousarray(b_v[sl]).reshape(P, 1),
                "wo": np.ascontiguousarray(w_o[sl, :]).astype(bf16),
            }
        )

    kwargs = {}
    if TRACE:
        import tempfile

        tdir = tempfile.mkdtemp(prefix="bass_trace_")
        kwargs["tmpdir"] = tdir
    res = run_bass_kernel_spmd(nc, in_maps, list(range(NCORES)), trace=TRACE, **kwargs)
    LAST_RESULT = {
        "exec_time_ns": res.exec_time_ns,
        "trace_path": (res.instructions_and_trace or (None, None))[1],
    }
    acc = np.zeros((L, D), np.float64)
    for i in range(NCORES):
        acc += res.results[i]["out"].astype(np.float64)
    acc += b_o.astype(np.float64)
    return acc.astype(np.float32).reshape(1, L, D)
